# revision 1
# baseline (speedup 1.0000x reference)
"""MinkowskiEngine deconv+ReLU v2: host-staged X stream + indicator-matmul
scatter, bf16 GEMM, no SWDGE.

Per core (output-partitioned, rows [c*rpc_eff, ...)):
  host sorts the core's K*M/8 pairs by (block, k, out-row); k-pure chunks of
  128 pairs; X = feats rows per chunk, pre-transposed to lhsT layout, streamed
  sequentially (no gather). Device: chunk GEMM (bf16) -> contrib PSUM ->
  ACT copy to SBUF bf16 -> indicator matmuls (lhsT=contrib, rhs=0/1 S built
  on DVE by is_equal(iota, srel)) accumulate out^T tiles in PSUM ->
  ACT bias+ReLU -> sequential DMA out. Host transposes/concats the output.

Structural schedule (chunk/instance counts) shared by all 8 cores; only
tensor contents differ (SPMD single program).
"""
import numpy as np
from contextlib import ExitStack

import concourse.bass as bass
import concourse.bacc as bacc
from concourse import mybir

P = 128
GG = 14          # tiles per block (block = PSUM-resident out^T stripe)
CH = 32          # chunks per X DMA tile (2 MB)
NXB = 3          # x stream buffers
NCSB = 4         # contrib sbuf ring (bank batches)
NSS = 16         # S sbuf slots (batches of SB instances)
SB = 8           # instances per S build
BCH = 4          # chunks per contrib PSUM bank batch
N_CORES = 8

# ---------------------------------------------------------------------------
# host schedule
# ---------------------------------------------------------------------------


def _windows(nb, n_t):
    return [((j * n_t) // nb, min(n_t - 1, ((j + 1) * n_t) // nb))
            for j in range(nb)]


def _greedy(d, ws):
    """d: per-tile pair counts. Returns (ok, takes) with takes[j]
    = [(tile, n), ...]."""
    d = list(d)
    takes = []
    for (lo, hi) in ws:
        room = P
        tk = []
        for t in range(lo, hi + 1):
            n = min(room, d[t])
            if n:
                tk.append((t, n))
                d[t] -= n
                room -= n
        takes.append(tk)
    return sum(d) == 0, takes


def build_schedule(in_map, out_map, n_in, n_out, n_cores=N_CORES):
    K, M = in_map.shape
    rpc_eff = -(-n_out // n_cores)
    n_tiles = -(-rpc_eff // P)
    assert n_tiles % GG == 0, (n_tiles, GG)
    nblk = n_tiles // GG
    rpc = n_tiles * P

    kk = np.repeat(np.arange(K, dtype=np.int64), M)
    irow = in_map.astype(np.int64).ravel()
    orow = out_map.astype(np.int64).ravel()
    core = np.minimum(orow // rpc_eff, n_cores - 1)

    percore = []
    for c in range(n_cores):
        m = core == c
        kc, ic, oc = kk[m], irow[m], orow[m] - c * rpc_eff
        blk = oc // (GG * P)
        order = np.lexsort((oc, kc, blk))
        percore.append((kc[order], ic[order], oc[order], blk[order]))

    # bucket counts and common NB
    cnt = np.zeros((n_cores, nblk, K), np.int64)
    for c in range(n_cores):
        kc, ic, oc, blk = percore[c]
        np.add.at(cnt[c], (blk, kc), 1)
    NB = np.maximum(-(-cnt.max(0) // P), 1)

    # per-core per-bucket tile histograms, bump NB until greedy feasible
    tilehist = np.zeros((n_cores, nblk, K, GG), np.int64)
    for c in range(n_cores):
        kc, ic, oc, blk = percore[c]
        tl = (oc - blk * GG * P) // P
        np.add.at(tilehist[c], (blk, kc, tl), 1)
    for b in range(nblk):
        for k in range(K):
            while True:
                ws = _windows(int(NB[b, k]), GG)
                if all(_greedy(tilehist[c, b, k], ws)[0]
                       for c in range(n_cores)):
                    break
                NB[b, k] += 1

    # shared chunk table, ordered (blk, window-pos, k) so early banks finish
    # early (per-bank relu can release PSUM for the next block sooner)
    chunks = []                      # (blk, k, lo, hi)
    chunk_idx = {}                   # (blk, k, j) -> chunk id
    for b in range(nblk):
        perk = [_windows(int(NB[b, k]), GG) for k in range(K)]
        order = sorted((perk[k][j][0], k, j)
                       for k in range(K) for j in range(len(perk[k])))
        for (lo0, k, j) in order:
            lo, hi = perk[k][j]
            chunk_idx[(b, k, j)] = len(chunks)
            chunks.append((b, k, lo, hi))
    nch = len(chunks)

    # shared instance table (for DVE S builds; None = padding) and merged
    # IND groups: one matmul per (chunk, psum-bank run), rhs = s consecutive
    # S tiles. Groups may not straddle an S-batch (SB) boundary.
    insts = []                       # (chunk_id, tile_local) | None
    groups = []                      # (chunk_id, t0, s, i0)
    for ci, (b, k, lo, hi) in enumerate(chunks):
        t = lo
        while t <= hi:
            t1 = min(hi, (t // 4) * 4 + 3)     # end of bank run
            s = t1 - t + 1
            if len(insts) % SB + s > SB:       # pad to S-batch boundary
                insts += [None] * (SB - len(insts) % SB)
            groups.append((ci, t, s, len(insts)))
            for tt in range(t, t1 + 1):
                insts.append((ci, tt))
            t = t1 + 1
    ninst = len(insts)
    # PSUM accumulation groups are per 2KB bank (start zeroes the whole
    # bank): flags keyed by (block, bank), over merged groups
    first = {}
    last = {}
    for g, (ci, t0, s, i0) in enumerate(groups):
        b = chunks[ci][0]
        key = (b, t0 // 4)
        if key not in first:
            first[key] = g
        last[key] = g
    for b in range(nblk):
        for t in range(GG):
            assert (b, t // 4) in first, ("uncovered bank", b, t)

    # per-core slot data: gidx [nch, P] feats row ids, orel_b [nch, P]
    # block-local out row (or -3000 for padding)
    per_core = []
    for c in range(n_cores):
        kc, ic, oc, blk = percore[c]
        gidx = np.zeros((nch, P), np.int64)
        orel = np.full((nch, P), -3000, np.int64)
        # bucket start offsets in the sorted pair list
        bstart = np.zeros((nblk, K), np.int64)
        np.cumsum(cnt[c].ravel()[:-1], out=bstart.ravel()[1:])
        for b in range(nblk):
            for k in range(K):
                ws = _windows(int(NB[b, k]), GG)
                ok, takes = _greedy(tilehist[c, b, k], ws)
                assert ok
                # per-tile read pointers within this bucket's sorted pairs
                toff = bstart[b, k] + np.concatenate(
                    ([0], np.cumsum(tilehist[c, b, k])[:-1]))
                ptr = toff.copy()
                for j, tk in enumerate(takes):
                    ci = chunk_idx[(b, k, j)]
                    pos = 0
                    for (t, n) in tk:
                        sl = slice(ptr[t], ptr[t] + n)
                        gidx[ci, pos:pos + n] = ic[sl]
                        orel[ci, pos:pos + n] = oc[sl] - b * GG * P
                        ptr[t] += n
                        pos += n
        per_core.append((gidx, orel))

    # per-group tight stream range: union over cores of rows the chunk
    # actually touches within the group's bank run (cols are rel to t0*128)
    grange = []
    for (ci, t0, s, i0) in groups:
        rmin, rmax = s * P, -1
        for (gidx, orel) in per_core:
            oc = orel[ci]
            m = (oc >= t0 * P) & (oc < (t0 + s) * P)
            if m.any():
                sel = oc[m] - t0 * P
                rmin = min(rmin, int(sel.min()))
                rmax = max(rmax, int(sel.max()))
        if rmax < 0:
            rmin, rmax = 0, 0
        grange.append((rmin, rmax - rmin + 1))

    return dict(NB=NB, chunks=chunks, insts=insts, groups=groups,
                first=first, last=last, grange=grange,
                nch=nch, ninst=ninst, nblk=nblk, n_tiles=n_tiles, rpc=rpc,
                rpc_eff=rpc_eff, K=K, per_core=per_core)


# ---------------------------------------------------------------------------
# input packing
# ---------------------------------------------------------------------------


def make_inputs(feats, weight, bias, sched):
    import ml_dtypes
    bf16 = ml_dtypes.bfloat16
    K = sched["K"]
    nch, ninst = sched["nch"], sched["ninst"]
    chunks, insts = sched["chunks"], sched["insts"]
    c_in = feats.shape[1]
    c_out = weight.shape[2]
    assert c_in == 256 and c_out == 128

    f16 = feats.astype(bf16)
    wd = np.ascontiguousarray(
        weight.astype(bf16).reshape(K, 2, P, c_out).transpose(2, 0, 1, 3)
    ).reshape(P, K * 2 * c_out)
    iota = np.tile(np.arange(P, dtype=np.int16), (P, SB))
    zd = np.zeros((P, 512), np.uint16)
    biasd = np.tile(bias.astype(np.float32)[:, None], (1, 8))

    nxt = -(-nch // CH)
    nip = -(-ninst // SB) * SB

    in_maps = []
    for (gidx, orel) in sched["per_core"]:
        # X: [nxt*128, CH*256]; [T*128+p, q*256+h*128+j] = f16[gidx[c,j], h*128+p]
        A = f16[gidx]                                  # [nch, j(P), 256]
        A = A.reshape(nch, P, 2, P).transpose(0, 3, 2, 1)   # [c, p, h, j]
        xpad = np.zeros((nxt * CH, P, 2, P), bf16)
        xpad[:nch] = A
        xd = np.ascontiguousarray(
            xpad.reshape(nxt, CH, P, 2, P).transpose(0, 2, 1, 3, 4)
        ).reshape(nxt * P, CH * 2 * P)
        # srel2: [P, nip*2] int16, col 2i,2i+1 = orel - 128*tile for inst i
        srel = np.full((nip, P), -3000, np.int64)
        for i, inst in enumerate(insts):
            if inst is not None:
                ci, t = inst
                srel[i] = orel[ci] - t * P
        srel2 = np.repeat(srel.T.astype(np.int16), 2, axis=1)
        in_maps.append(dict(xd=xd.view(np.uint16), wd=wd.view(np.uint16),
                            srel2=srel2, iota=iota, biasd=biasd, zd=zd))
    return in_maps


# ---------------------------------------------------------------------------
# device program
# ---------------------------------------------------------------------------


def build_program(sched):
    K = sched["K"]
    nch, ninst, nblk = sched["nch"], sched["ninst"], sched["nblk"]
    chunks, insts = sched["chunks"], sched["insts"]
    groups = sched["groups"]
    grange = sched["grange"]
    first, last = sched["first"], sched["last"]
    rpc = sched["rpc"]
    c_out = 128
    nxt = -(-nch // CH)
    nip = -(-ninst // SB) * SB
    nsb = nip // SB
    nbat = -(-nch // BCH)
    BW = GG * P                    # block out^T width (1792)

    grp_of_chunk = [[] for _ in range(nch)]
    for g, (ci, t0, s, i0) in enumerate(groups):
        grp_of_chunk[ci].append(g)

    # ---- symbolic engine sequences for position bookkeeping ----
    # PE: per batch a: MMs (2 per chunk), then merged INDs of batch a-1
    pe_seq = []
    for a in range(nbat + 1):
        if a < nbat:
            for c in range(a * BCH, min((a + 1) * BCH, nch)):
                pe_seq.append(("MM", c))           # one entry = 2 matmuls
        if a >= 1:
            lo, hi = (a - 1) * BCH, min(a * BCH, nch)
            for ci in range(lo, hi):
                for g in grp_of_chunk[ci]:
                    cig, t0g = groups[g][0], groups[g][1]
                    keyg = (chunks[cig][0], t0g // 4)
                    if first[keyg] == g:
                        pe_seq.append(("Z", keyg))
                    pe_seq.append(("IND", g))
    pe_pos = {}
    pos = 0
    for e in pe_seq:
        pos += 1                     # pe_sem: +1 per MM pair, +1 per IND
        pe_pos[e] = pos              # sem value after entry completes
    pe_total = pos

    pe_pos_mm_hi = {}                # chunk -> pos
    pe_pos_ind = {}                  # group -> pos
    for e in pe_seq:
        if e[0] == "MM":
            pe_pos_mm_hi[e[1]] = pe_pos[e]
        else:
            pe_pos_ind[e[1]] = pe_pos[e]

    # ACT: per batch a: CP(a); RELU(b, bank) as soon as that bank's last IND
    # group has been emitted (its group is in PE iteration batch(ci)+1)
    relu_bat = {}
    for (b, bank), g in last.items():
        relu_bat[(b, bank)] = min(groups[g][0] // BCH + 1, nbat)
    act_seq = []
    for a in range(nbat + 1):
        if a < nbat:
            act_seq.append(("CP", a))
        for (b, bank), rb in sorted(relu_bat.items()):
            if rb == a:
                act_seq.append(("RELU", b, bank))
    act_pos = {}
    pos = 0
    for e in act_seq:
        pos += 1
        act_pos[e] = pos
    act_pos_cp = {e[1]: act_pos[e] for e in act_seq if e[0] == "CP"}
    act_pos_relu = {}                # block -> pos of last relu
    act_pos_relu_bank = {}           # (block, bank) -> pos
    for e in act_seq:
        if e[0] == "RELU":
            act_pos_relu[e[1]] = max(act_pos_relu.get(e[1], 0), act_pos[e])
            act_pos_relu_bank[(e[1], e[2])] = act_pos[e]
    act_total = pos

    # DVE consumer positions: last IND group consuming S batch sb
    sbatch_last_ind = {}
    for g, (ci, t0, s, i0) in enumerate(groups):
        sb = i0 // SB
        sbatch_last_ind[sb] = max(sbatch_last_ind.get(sb, 0), pe_pos_ind[g])
    for sb in range(nsb):            # padding-only batches: no consumer
        if sb not in sbatch_last_ind:
            sbatch_last_ind[sb] = 0

    # S builds all on DVE (walrus rejects tensor_tensor on Pool)
    on_dve = [True for sb in range(nsb)]
    eng_cum = []                     # sb -> (engine_idx, count-within-engine)
    ndve = ngp = 0
    for sb in range(nsb):
        if on_dve[sb]:
            ndve += 1
            eng_cum.append((0, ndve))
        else:
            ngp += 1
            eng_cum.append((1, ngp))
    # highest S batch needed by each block's IND groups
    sb_hi_blk = {}
    for g, (ci, t0, s, i0) in enumerate(groups):
        b = chunks[ci][0]
        sb_hi_blk[b] = max(sb_hi_blk.get(b, 0), (i0 + s - 1) // SB)

    # out-dma count before block b's osb slot (b%2) is free
    nc = bacc.Bacc("TRN2", target_bir_lowering=False, debug=False)
    xd_t = nc.dram_tensor("xd", [nxt * P, CH * 2 * P], mybir.dt.uint16,
                          kind="ExternalInput").ap()
    wd_t = nc.dram_tensor("wd", [P, K * 2 * c_out], mybir.dt.uint16,
                          kind="ExternalInput").ap()
    srel_t = nc.dram_tensor("srel2", [P, nip * 2], mybir.dt.int16,
                            kind="ExternalInput").ap()
    iota_t = nc.dram_tensor("iota", [P, SB * P], mybir.dt.int16,
                            kind="ExternalInput").ap()
    bias_t = nc.dram_tensor("biasd", [P, 8], mybir.dt.float32,
                            kind="ExternalInput").ap()
    zd_t = nc.dram_tensor("zd", [P, 512], mybir.dt.uint16,
                          kind="ExternalInput").ap()
    outT = nc.dram_tensor("outT", [P, rpc], mybir.dt.bfloat16,
                          kind="ExternalOutput").ap()

    SRA = max(1, min(nsb, 64))       # S batches covered by the first srel dma

    with ExitStack() as stack:
        block = stack.enter_context(nc.Block())
        wld_sem = stack.enter_context(nc.semaphore("wld"))
        srl_sem = stack.enter_context(nc.semaphore("srl"))
        srlb_sem = stack.enter_context(nc.semaphore("srlb"))
        io_sem = stack.enter_context(nc.semaphore("io"))
        bs_sem = stack.enter_context(nc.semaphore("bs"))
        x_sems = [stack.enter_context(nc.semaphore(f"x{i}"))
                  for i in range(NXB)]
        pe_sem = stack.enter_context(nc.semaphore("pe"))
        act_sem = stack.enter_context(nc.semaphore("act"))
        dve_sem = stack.enter_context(nc.semaphore("dve"))
        out_sems = [stack.enter_context(nc.semaphore(f"out{i}"))
                    for i in range(2)]

        # PSUM: ops banks 0-3 (out^T stripe), cps banks 4,5 (contrib)
        ops = stack.enter_context(
            nc.psum_tensor("ops", [P, 4, 512], mybir.dt.float32))
        cps = stack.enter_context(
            nc.psum_tensor("cps", [P, 2, BCH, c_out], mybir.dt.float32))

        x_sb = stack.enter_context(
            nc.sbuf_tensor("x_sb", [P, NXB, CH * 2 * P], mybir.dt.bfloat16))
        w_sb = stack.enter_context(
            nc.sbuf_tensor("w_sb", [P, K * 2 * c_out], mybir.dt.bfloat16))
        srel_sb = stack.enter_context(
            nc.sbuf_tensor("srel_sb", [P, nip * 2], mybir.dt.int16))
        iota_sb = stack.enter_context(
            nc.sbuf_tensor("iota_sb", [P, SB * P], mybir.dt.int16))
        bias_sb = stack.enter_context(
            nc.sbuf_tensor("bias_sb", [P, 8], mybir.dt.float32))
        zero_sb = stack.enter_context(
            nc.sbuf_tensor("zero_sb", [P, 512], mybir.dt.bfloat16))
        s_sb = stack.enter_context(
            nc.sbuf_tensor("s_sb", [P, NSS, SB * P], mybir.dt.bfloat16))
        c_sb = stack.enter_context(
            nc.sbuf_tensor("c_sb", [P, NCSB, BCH * c_out], mybir.dt.bfloat16))
        o_sb = stack.enter_context(
            nc.sbuf_tensor("o_sb", [P, 2, BW], mybir.dt.bfloat16))

        @block.sync
        def _(sy):
            def x_tile(T):
                sy.dma_start(out=x_sb[:, T % NXB, :].bitcast(mybir.dt.uint16),
                             in_=xd_t[T * P:(T + 1) * P, :]
                             ).then_inc(x_sems[T % NXB], 16)

            # critical-path order: weights + first x tiles first, bulk srel
            # later (PE can start ~20us earlier)
            ca = SRA * SB * 2
            sy.dma_start(out=w_sb[:].bitcast(mybir.dt.uint16),
                         in_=wd_t[:]).then_inc(wld_sem, 16)
            sy.dma_start(out=zero_sb[:].bitcast(mybir.dt.uint16),
                         in_=zd_t[:]).then_inc(wld_sem, 16)
            x_tile(0)
            sy.dma_start(out=srel_sb[:, :ca],
                         in_=srel_t[:, :ca]).then_inc(srl_sem, 16)
            sy.dma_start(out=iota_sb[:], in_=iota_t[:]).then_inc(io_sem, 16)
            if nxt > 1:
                x_tile(1)
            sy.dma_start(out=bias_sb[:], in_=bias_t[:]).then_inc(bs_sem, 16)
            if nxt > 2:
                x_tile(2)
            if ca < nip * 2:
                sy.dma_start(out=srel_sb[:, ca:],
                             in_=srel_t[:, ca:]).then_inc(srlb_sem, 16)
            for T in range(NXB, nxt):
                lc = min((T - NXB + 1) * CH, nch) - 1
                sy.wait_ge(pe_sem, pe_pos_mm_hi[lc])
                x_tile(T)

        def s_build(eng, sem, sb):
            m = min(SB, max(1, ninst - sb * SB))
            in1 = bass.AP(srel_sb.ap().tensor, sb * SB * 2,
                          [[nip * 2, P], [2, m], [0, P // 2], [1, 2]])
            eng.tensor_tensor(out=s_sb[:, sb % NSS, :m * P],
                              in0=iota_sb[:, :m * P], in1=in1,
                              op=mybir.AluOpType.is_equal).then_inc(sem, 1)

        @block.gpsimd
        def _(gp):
            for b in range(nblk):
                gp.wait_ge(act_sem, act_pos_relu[b])
                gp.dma_start(out=outT[:, b * BW:(b + 1) * BW],
                             in_=o_sb[:, b % 2, :]).then_inc(out_sems[b % 2], 16)

        @block.tensor
        def _(pe):
            pe.wait_ge(wld_sem, 32)
            for (op, idx) in pe_seq:
                if op == "MM":
                    c = idx
                    b, k, lo, hi = chunks[c]
                    a = c // BCH
                    T = c // CH
                    if c % CH == 0:
                        pe.wait_ge(x_sems[T % NXB], 16 * (T // NXB + 1))
                    if c % BCH == 0 and a >= 2:
                        pe.wait_ge(act_sem, act_pos_cp[a - 2])
                    q = c % BCH
                    qlast = min((a + 1) * BCH, nch) - a * BCH - 1
                    xoff = (c % CH) * 2 * P
                    pe.matmul(out=cps[:, a % 2, q, :],
                              lhsT=x_sb[:, T % NXB, xoff:xoff + P],
                              rhs=w_sb[:, (k * 2) * c_out:(k * 2 + 1) * c_out],
                              start=(q == 0), stop=False)
                    pe.matmul(out=cps[:, a % 2, q, :],
                              lhsT=x_sb[:, T % NXB, xoff + P:xoff + 2 * P],
                              rhs=w_sb[:, (k * 2 + 1) * c_out:(k * 2 + 2) * c_out],
                              start=False, stop=(q == qlast)).then_inc(pe_sem, 1)
                elif op == "Z":
                    b, bank = idx
                    if b >= 1:
                        pe.wait_ge(act_sem, act_pos_relu_bank[(b - 1, bank)])
                    pe.matmul(out=ops[:, bank, 0:512],
                              lhsT=w_sb[:, 0:P], rhs=zero_sb[:, 0:512],
                              start=True, stop=False).then_inc(pe_sem, 1)
                else:
                    g = idx
                    ci, t0, s, i0 = groups[g]
                    b = chunks[ci][0]
                    a = ci // BCH
                    sb = i0 // SB
                    bank = t0 // 4
                    pe.wait_ge(act_sem, act_pos_cp[a])
                    pe.wait_ge(dve_sem, sb + 1)
                    off, w = grange[g]
                    col = (t0 % 4) * P + off
                    pe.matmul(out=ops[:, bank, col:col + w],
                              lhsT=c_sb[:, a % NCSB,
                                        (ci % BCH) * c_out:(ci % BCH + 1) * c_out],
                              rhs=s_sb[:, sb % NSS,
                                       (i0 % SB) * P + off:(i0 % SB) * P + off + w],
                              start=False,
                              stop=(last[(b, bank)] == g),
                              ).then_inc(pe_sem, 1)

        @block.scalar
        def _(sc):
            sc.wait_ge(bs_sem, 16)
            for e in act_seq:
                if e[0] == "CP":
                    a = e[1]
                    n = min((a + 1) * BCH, nch) - a * BCH
                    sc.wait_ge(pe_sem, pe_pos_mm_hi[a * BCH + n - 1])
                    sc.copy(out=c_sb[:, a % NCSB, :n * c_out],
                            in_=cps[:, a % 2, 0:n, :]).then_inc(act_sem, 1)
                else:
                    _, b, bank = e
                    sc.wait_ge(pe_sem, pe_pos_ind[last[(b, bank)]])
                    if b >= 2:
                        sc.wait_ge(out_sems[b % 2], 16 * (b // 2))
                    w0 = bank * 512
                    w1 = min(BW, w0 + 512)
                    sc.activation(out=o_sb[:, b % 2, w0:w1],
                                  in_=ops[:, bank, 0:w1 - w0],
                                  func=mybir.ActivationFunctionType.Relu,
                                  bias=bias_sb[:, 0:1], scale=1.0
                                  ).then_inc(act_sem, 1)

        @block.vector
        def _(ve):
            ve.wait_ge(io_sem, 16)
            ve.wait_ge(srl_sem, 16)
            for sb in range(nsb):
                if sb == SRA:
                    ve.wait_ge(srlb_sem, 16)
                if sb >= NSS:
                    ve.wait_ge(pe_sem, sbatch_last_ind[sb - NSS])
                s_build(ve, dve_sem, sb)

    nc.compile()
    return nc


# ---------------------------------------------------------------------------
# entry
# ---------------------------------------------------------------------------

_CACHE = {}


def kernel(feats, weight, bias, in_map, out_map, n_out):
    from concourse.bass_utils import run_bass_kernel_spmd

    feats = np.asarray(feats, dtype=np.float32)
    weight = np.asarray(weight, dtype=np.float32)
    bias = np.asarray(bias, dtype=np.float32)
    in_map = np.asarray(in_map)
    out_map = np.asarray(out_map)
    n_out = int(n_out)
    n_in = feats.shape[0]
    K = weight.shape[0]

    sched = build_schedule(in_map, out_map, n_in, n_out, N_CORES)
    in_maps = make_inputs(feats, weight, bias, sched)

    key = (n_in, n_out, K, sched["nch"], sched["ninst"])
    nc = _CACHE.get(key)
    if nc is None:
        nc = build_program(sched)
        _CACHE[key] = nc

    res = run_bass_kernel_spmd(nc, in_maps, list(range(N_CORES)))
    rpc_eff = sched["rpc_eff"]
    outs = []
    for c in range(N_CORES):
        r = min(rpc_eff, n_out - c * rpc_eff)
        ot = res.results[c]["outT"]              # [128, rpc] bf16
        outs.append(np.asarray(ot[:, :r], dtype=np.float32).T)
    return np.ascontiguousarray(np.concatenate(outs, 0))



# revision 24
# speedup vs baseline: 1.1417x; 1.1417x over previous
"""MinkowskiEngine deconv+ReLU v3: breakpoint row-windows + span-packed S.

Per core (output-partitioned, rows [c*rpc_eff, ...)):
  host sorts the core's K*M/8 pairs by (block, k, out-row). For each
  (block, k) bucket, shared row BREAKPOINTS close a window when any core
  would exceed 128 pairs; chunk = (b, k, window) holds <=128 pairs per
  core (k-pure, row-range-pure). X = feats rows per chunk slot,
  pre-transposed to lhsT layout, streamed sequentially (no gather).

Device: chunk GEMM (bf16) -> contrib PSUM -> ACT copy to SBUF bf16 ->
  indicator matmuls (lhsT=contrib, rhs=0/1 S) accumulate out^T tiles in
  PSUM -> ACT bias+ReLU -> out DMA (ACT queue). S matrices are built
  span-packed: one DVE/GpSimd is_equal op per batch of SGB groups, each
  group's columns sized to the batch max width (vs per-tile 128-wide
  instances) -- ~40% less vector-engine work.

Structural schedule (chunk/group counts) shared by all 8 cores; only
tensor contents differ (SPMD single program).
"""
import numpy as np
from contextlib import ExitStack

import concourse.bass as bass
import concourse.bacc as bacc
from concourse import mybir

P = 128
GG = 14          # tiles per block (block = PSUM-resident out^T stripe)
CH = 32          # chunks per X DMA tile (2 MB)
NXB = 4          # x stream buffers
NCSB = 4         # contrib sbuf ring (bank batches)
BCH = 4          # chunks per contrib PSUM bank batch
SGB = 16         # groups per S build batch
SORT_WIN = 16    # width-sort window; = SGB keeps build order == consume
                 # order (wider windows overflow the S ring -> deadlock)
SRING = 16384    # S ring columns (bf16)
N_CORES = 8

# ---------------------------------------------------------------------------
# host schedule
# ---------------------------------------------------------------------------


def build_schedule(in_map, out_map, n_in, n_out, n_cores=N_CORES):
    K, M = in_map.shape
    rpc_eff = -(-n_out // n_cores)
    n_tiles = -(-rpc_eff // P)
    assert n_tiles % GG == 0, (n_tiles, GG)
    nblk = n_tiles // GG
    rpc = n_tiles * P
    R = GG * P                        # rows per block (1792)

    kk = np.repeat(np.arange(K, dtype=np.int64), M)
    irow = in_map.astype(np.int64).ravel()
    orow = out_map.astype(np.int64).ravel()
    core = np.minimum(orow // rpc_eff, n_cores - 1)

    percore = []                      # (kc, ic, oc) sorted by (blk, k, oc)
    cnt = np.zeros((n_cores, nblk, K), np.int64)
    for c in range(n_cores):
        m = core == c
        kc, ic, oc = kk[m], irow[m], orow[m] - c * rpc_eff
        blk = oc // R
        order = np.lexsort((oc, kc, blk))
        percore.append((kc[order], ic[order], oc[order]))
        np.add.at(cnt[c], (blk, kc), 1)

    # bucket start offsets in each core's sorted pair list
    bstart = np.zeros((n_cores, nblk * K), np.int64)
    for c in range(n_cores):
        np.cumsum(cnt[c].reshape(-1)[:-1], out=bstart[c, 1:])
    bstart = bstart.reshape(n_cores, nblk, K)

    # shared row breakpoints per (b, k): close window when any core would
    # exceed P pairs. windows[b][k] = list of (r_lo, r_hi) block-local rows.
    # chunks table in processing order (b, r_lo, k).
    chunks = []                       # (b, k, r_lo, r_hi, [per-core slices])
    for b in range(nblk):
        blk_chunks = []
        for k in range(K):
            # per-core row lists for this bucket
            rows_c = []
            for c in range(n_cores):
                s0 = bstart[c, b, k]
                rows_c.append(percore[c][2][s0:s0 + cnt[c, b, k]] - b * R)
            n_c = [len(r) for r in rows_c]
            pos = [0] * n_cores
            while any(pos[c] < n_c[c] for c in range(n_cores)):
                # shared row frontier: the row of the (P+1)-th pending pair,
                # minimized over cores. Each core then takes up to P pairs
                # with row <= frontier (boundary-row pairs may split across
                # windows, keeping the binding core exactly full).
                r_hi = R
                for c in range(n_cores):
                    rem = n_c[c] - pos[c]
                    if rem > P:
                        r_hi = min(r_hi, int(rows_c[c][pos[c] + P]))
                ends = []
                w_lo, w_hi = R, 0
                for c in range(n_cores):
                    e = pos[c] + min(P, int(np.searchsorted(
                        rows_c[c][pos[c]:], r_hi, side="right")))
                    if e > pos[c]:
                        w_lo = min(w_lo, int(rows_c[c][pos[c]]))
                        w_hi = max(w_hi, int(rows_c[c][e - 1]) + 1)
                    ends.append(e)
                assert w_hi > 0
                blk_chunks.append((b, k, w_lo, w_hi,
                                   [(pos[c], ends[c]) for c in range(n_cores)]))
                pos = ends
        blk_chunks.sort(key=lambda t: (t[2], t[1]))
        chunks.extend(blk_chunks)
    nch = len(chunks)

    # groups: chunk clipped to PSUM bank (512-col) ranges, tight union range
    groups = []                       # (ci, bank, rmin, w)
    gsrel = []                        # per group: list over cores of
                                      # (slot_positions, srel_values)
    for ci, (b, k, r_lo, r_hi, sl) in enumerate(chunks):
        bank_lo = r_lo // 512
        bank_hi = (r_hi - 1) // 512
        for bank in range(bank_lo, bank_hi + 1):
            lo = max(r_lo, bank * 512)
            hi = min(r_hi, (bank + 1) * 512)
            rmin, rmax = 1 << 30, -1
            percore_part = []
            for c in range(n_cores):
                s0 = bstart[c, b, k]
                p0, p1 = sl[c]
                rows = percore[c][2][s0 + p0:s0 + p1] - b * R
                m = (rows >= lo) & (rows < hi)
                if m.any():
                    rr = rows[m]
                    rmin = min(rmin, int(rr.min()))
                    rmax = max(rmax, int(rr.max()))
                percore_part.append(m)
            if rmax < 0:
                continue
            w = rmax - rmin + 1
            if w & 1:                 # even width for the srel x2 AP trick
                if rmax + 1 < (bank + 1) * 512:
                    w += 1
                else:                 # at bank end: extend left instead
                    rmin -= 1
                    w += 1
            gs = []
            for c in range(n_cores):
                s0 = bstart[c, b, k]
                p0, p1 = sl[c]
                rows = percore[c][2][s0 + p0:s0 + p1] - b * R
                m = percore_part[c]
                gs.append((np.nonzero(m)[0], rows[m] - rmin))
            groups.append((ci, bank, rmin, w))
            gsrel.append(gs)
    ngroups = len(groups)

    # S build batches: SGB groups of similar width from a SORT_WIN sliding
    # window (cuts pad-to-max waste); srel columns stored in BUILD order so
    # each batch reads a contiguous srel2 slice. Ring allocation with wrap;
    # per-batch blocker = max group index whose IND must complete first
    # (pe_pos_ind is monotone in group index).
    build_order = []
    for w0 in range(0, ngroups, SORT_WIN):
        idx = sorted(range(w0, min(w0 + SORT_WIN, ngroups)),
                     key=lambda g: groups[g][3])
        build_order.extend(idx)
    nbatch = -(-ngroups // SGB)
    batch_groups = [build_order[j * SGB:(j + 1) * SGB] for j in range(nbatch)]
    batch_w = [max(groups[g][3] for g in bg) for bg in batch_groups]
    batch_off = []
    batch_blocker = []                # group index that must be consumed
    placed = []                       # (start, end, last_group)
    off = 0
    for j in range(nbatch):
        wb = batch_w[j]
        sz = SGB * wb
        assert sz <= SRING, (j, wb)
        if off + sz > SRING:
            off = 0
        s, e = off, off + sz
        blocker = -1
        for (ps, pe_, lg) in placed:
            if ps < e and s < pe_:
                blocker = max(blocker, lg)
        placed = [(ps, pe_, lg) for (ps, pe_, lg) in placed if lg > blocker]
        placed.append((s, e, max(batch_groups[j])))
        batch_off.append(off)
        batch_blocker.append(blocker)
        off = e

    # group -> batch, S column start, srel build position
    g_batch = np.empty(ngroups, np.int64)
    g_scol = np.empty(ngroups, np.int64)
    g_bp = np.empty(ngroups, np.int64)
    for j, bg in enumerate(batch_groups):
        for i, g in enumerate(bg):
            g_batch[g] = j
            g_scol[g] = batch_off[j] + i * batch_w[j]
            g_bp[g] = j * SGB + i

    # PSUM accumulation bookkeeping per (block, bank)
    first = {}
    last = {}
    for g, (ci, bank, rmin, w) in enumerate(groups):
        b = chunks[ci][0]
        key = (b, bank)
        if key not in first:
            first[key] = g
        last[key] = g
    for b in range(nblk):
        for t in range(4):
            assert (b, t) in first, ("uncovered bank", b, t)

    grp_of_chunk = [[] for _ in range(nch)]
    for g, (ci, bank, rmin, w) in enumerate(groups):
        grp_of_chunk[ci].append(g)

    return dict(chunks=chunks, groups=groups, gsrel=gsrel, g_scol=g_scol,
                g_batch=g_batch, g_bp=g_bp,
                batch_w=batch_w, batch_off=batch_off,
                batch_blocker=batch_blocker, nbatch=nbatch,
                first=first, last=last, grp_of_chunk=grp_of_chunk,
                nch=nch, ngroups=ngroups, nblk=nblk, n_tiles=n_tiles,
                rpc=rpc, rpc_eff=rpc_eff, K=K,
                percore=percore, bstart=bstart, cnt=cnt)


# ---------------------------------------------------------------------------
# input packing
# ---------------------------------------------------------------------------


def make_inputs(feats, weight, bias, sched):
    import ml_dtypes
    bf16 = ml_dtypes.bfloat16
    K = sched["K"]
    nch, ngroups = sched["nch"], sched["ngroups"]
    chunks, groups, gsrel = sched["chunks"], sched["groups"], sched["gsrel"]
    percore, bstart = sched["percore"], sched["bstart"]
    c_in = feats.shape[1]
    c_out = weight.shape[2]
    assert c_in == 256 and c_out == 128

    f16 = feats.astype(bf16)
    wd = np.ascontiguousarray(
        weight.astype(bf16).reshape(K, 2, P, c_out).transpose(2, 0, 1, 3)
    ).reshape(P, K * 2 * c_out)
    iota = np.tile(np.arange(512, dtype=np.int16), (P, 1)).copy()
    zd = np.zeros((P, 512), np.uint16)
    biasd = np.tile(bias.astype(np.float32)[:, None], (1, 8))

    nxt = -(-nch // CH)

    in_maps = []
    for c in range(N_CORES):
        kc, ic, oc = percore[c]
        gidx = np.zeros((nch, P), np.int64)
        for ci, (b, k, r_lo, r_hi, sl) in enumerate(chunks):
            s0 = bstart[c, b, k]
            p0, p1 = sl[c]
            n = p1 - p0
            if n:
                gidx[ci, :n] = ic[s0 + p0:s0 + p1]
        # X: [nxt*128, CH*256]; [T*128+p, q*256+h*128+j] = f16[gidx[c,j], h*128+p]
        A = f16[gidx]                                  # [nch, j(P), 256]
        A = A.reshape(nch, P, 2, P).transpose(0, 3, 2, 1)   # [c, p, h, j]
        xpad = np.zeros((nxt * CH, P, 2, P), bf16)
        xpad[:nch] = A
        xd = np.ascontiguousarray(
            xpad.reshape(nxt, CH, P, 2, P).transpose(0, 2, 1, 3, 4)
        ).reshape(nxt * P, CH * 2 * P)
        # srel2: [P, 2*nbp] int16 in BUILD order; cols 2bp,2bp+1 = row
        # offset of the pair within its group (or -3000)
        g_bp = sched["g_bp"]
        nbp = sched["nbatch"] * SGB
        srel = np.full((nbp, P), -3000, np.int64)
        for g in range(ngroups):
            slot_pos, vals = gsrel[g][c]
            if len(slot_pos):
                srel[g_bp[g], slot_pos] = vals
        srel2 = np.repeat(srel.T.astype(np.int16), 2, axis=1)
        in_maps.append(dict(xd=xd.view(np.uint16), wd=wd.view(np.uint16),
                            srel2=srel2, iota=iota, biasd=biasd, zd=zd))
    return in_maps


# ---------------------------------------------------------------------------
# device program
# ---------------------------------------------------------------------------


def build_program(sched):
    K = sched["K"]
    nch, ngroups, nblk = sched["nch"], sched["ngroups"], sched["nblk"]
    chunks, groups = sched["chunks"], sched["groups"]
    g_scol, g_batch = sched["g_scol"], sched["g_batch"]
    batch_w, batch_off = sched["batch_w"], sched["batch_off"]
    batch_blocker, nbatch = sched["batch_blocker"], sched["nbatch"]
    first, last = sched["first"], sched["last"]
    grp_of_chunk = sched["grp_of_chunk"]
    rpc = sched["rpc"]
    c_out = 128
    nxt = -(-nch // CH)
    nbat = -(-nch // BCH)
    nbp = nbatch * SGB
    BW = GG * P                    # block out^T width (1792)

    # ACT: per batch a: CP(a); RELU(b, bank) as soon as that bank's last IND
    # group has been emitted; out DMA of block b after its last RELU.
    relu_bat = {}
    for (b, bank), g in last.items():
        relu_bat[(b, bank)] = min(groups[g][0] // BCH + 1, nbat)
    blk_last_bank = {}               # block -> (bat, bank) of its last relu
    act_seq = []
    for a in range(nbat + 1):
        if a < nbat:
            act_seq.append(("CP", a))
        ready = sorted((b, bank) for (b, bank), rb in relu_bat.items()
                       if rb == a)
        done_blocks = []
        for (b, bank) in ready:
            act_seq.append(("RELU", b, bank))
            blk_last_bank.setdefault(b, 0)
            blk_last_bank[b] += 1
            if blk_last_bank[b] == 4:
                done_blocks.append(b)
        for b in sorted(done_blocks):
            act_seq.append(("OUT", b))
    act_pos = {}
    pos = 0
    for e in act_seq:
        if e[0] != "OUT":            # OUT does not inc act_sem
            pos += 1
        act_pos[e] = pos
    act_pos_cp = {e[1]: act_pos[e] for e in act_seq if e[0] == "CP"}
    act_pos_relu_bank = {}           # (block, bank) -> pos
    for e in act_seq:
        if e[0] == "RELU":
            act_pos_relu_bank[(e[1], e[2])] = act_pos[e]


    # ---- symbolic engine sequences for position bookkeeping ----
    # PE: per batch a: MMs (2 per chunk), then merged INDs of batch a-1
    pe_seq = []
    for a in range(nbat + 1):
        if a < nbat:
            for c in range(a * BCH, min((a + 1) * BCH, nch)):
                pe_seq.append(("MM", c))           # one entry = 2 matmuls
        if a >= 1:
            lo, hi = (a - 1) * BCH, min(a * BCH, nch)
            batch_grps = [g for ci in range(lo, hi) for g in grp_of_chunk[ci]]
            if batch_grps:
                # hoisted waits: one dve + one act wait for the whole IND
                # run so the PE reorder window can pull LDWEIGHTS ahead of
                # in-flight matmuls (per-matmul waits serialize LDW).
                # act value covers the CP and any Z's relu-free precondition
                # (act_sem is monotone).
                zkeys = []
                for g in batch_grps:
                    keyg = (chunks[groups[g][0]][0], groups[g][1])
                    if first[keyg] == g and keyg[0] >= 1:
                        zkeys.append((keyg[0] - 1, keyg[1]))
                actv = max([act_pos_cp[a - 1]] +
                           [act_pos_relu_bank[zk] for zk in zkeys])
                dvev = max(int(g_batch[g]) + 1 for g in batch_grps)
                pe_seq.append(("W", (actv, dvev)))
            for ci in range(lo, hi):
                for g in grp_of_chunk[ci]:
                    cig, bankg = groups[g][0], groups[g][1]
                    keyg = (chunks[cig][0], bankg)
                    if first[keyg] == g:
                        pe_seq.append(("Z", keyg))
                    pe_seq.append(("IND", g))
    pe_pos = {}
    pos = 0
    for e in pe_seq:
        if e[0] != "W":              # W = wait-only, no pe_sem inc
            pos += 1                 # pe_sem: +1 per MM pair, +1 per IND/Z
        pe_pos[e] = pos
    pe_pos_mm_hi = {}                # chunk -> pos
    pe_pos_ind = {}                  # group -> pos
    for e in pe_seq:
        if e[0] == "MM":
            pe_pos_mm_hi[e[1]] = pe_pos[e]
        elif e[0] == "IND":
            pe_pos_ind[e[1]] = pe_pos[e]

    # S-batch needed by IND group g: batch(g); builder blocker positions
    # SRA: batches covered by the first srel dma
    SRA = max(1, min(nbatch, 64))

    nc = bacc.Bacc("TRN2", target_bir_lowering=False, debug=False)
    xd_t = nc.dram_tensor("xd", [nxt * P, CH * 2 * P], mybir.dt.uint16,
                          kind="ExternalInput").ap()
    wd_t = nc.dram_tensor("wd", [P, K * 2 * c_out], mybir.dt.uint16,
                          kind="ExternalInput").ap()
    srel_t = nc.dram_tensor("srel2", [P, nbp * 2], mybir.dt.int16,
                            kind="ExternalInput").ap()
    iota_t = nc.dram_tensor("iota", [P, 512], mybir.dt.int16,
                            kind="ExternalInput").ap()
    bias_t = nc.dram_tensor("biasd", [P, 8], mybir.dt.float32,
                            kind="ExternalInput").ap()
    zd_t = nc.dram_tensor("zd", [P, 512], mybir.dt.uint16,
                          kind="ExternalInput").ap()
    outT = nc.dram_tensor("outT", [P, rpc], mybir.dt.bfloat16,
                          kind="ExternalOutput").ap()

    with ExitStack() as stack:
        block = stack.enter_context(nc.Block())
        wld_sem = stack.enter_context(nc.semaphore("wld"))
        srl_sem = stack.enter_context(nc.semaphore("srl"))
        srlb_sem = stack.enter_context(nc.semaphore("srlb"))
        io_sem = stack.enter_context(nc.semaphore("io"))
        bs_sem = stack.enter_context(nc.semaphore("bs"))
        x_sems = [stack.enter_context(nc.semaphore(f"x{i}"))
                  for i in range(NXB)]
        pe_sem = stack.enter_context(nc.semaphore("pe"))
        act_sem = stack.enter_context(nc.semaphore("act"))
        dve_sem = stack.enter_context(nc.semaphore("dve"))
        out_sems = [stack.enter_context(nc.semaphore(f"out{i}"))
                    for i in range(2)]

        # PSUM: ops banks 0-3 (out^T stripe), cps banks 4,5 (contrib)
        ops = stack.enter_context(
            nc.psum_tensor("ops", [P, 4, 512], mybir.dt.float32))
        cps = stack.enter_context(
            nc.psum_tensor("cps", [P, 2, BCH, c_out], mybir.dt.float32))

        x_sb = stack.enter_context(
            nc.sbuf_tensor("x_sb", [P, NXB, CH * 2 * P], mybir.dt.bfloat16))
        w_sb = stack.enter_context(
            nc.sbuf_tensor("w_sb", [P, K * 2 * c_out], mybir.dt.bfloat16))
        srel_sb = stack.enter_context(
            nc.sbuf_tensor("srel_sb", [P, nbp * 2], mybir.dt.int16))
        iota_sb = stack.enter_context(
            nc.sbuf_tensor("iota_sb", [P, 512], mybir.dt.int16))
        bias_sb = stack.enter_context(
            nc.sbuf_tensor("bias_sb", [P, 8], mybir.dt.float32))
        zero_sb = stack.enter_context(
            nc.sbuf_tensor("zero_sb", [P, 512], mybir.dt.bfloat16))
        s_sb = stack.enter_context(
            nc.sbuf_tensor("s_sb", [P, SRING], mybir.dt.bfloat16))
        c_sb = stack.enter_context(
            nc.sbuf_tensor("c_sb", [P, NCSB, BCH * c_out], mybir.dt.bfloat16))
        o_sb = stack.enter_context(
            nc.sbuf_tensor("o_sb", [P, 2, BW], mybir.dt.bfloat16))

        @block.sync
        def _(sy):
            def x_tile(T):
                sy.dma_start(out=x_sb[:, T % NXB, :].bitcast(mybir.dt.uint16),
                             in_=xd_t[T * P:(T + 1) * P, :]
                             ).then_inc(x_sems[T % NXB], 16)

            ca = SRA * SGB * 2
            sy.dma_start(out=w_sb[:].bitcast(mybir.dt.uint16),
                         in_=wd_t[:]).then_inc(wld_sem, 16)
            sy.dma_start(out=zero_sb[:].bitcast(mybir.dt.uint16),
                         in_=zd_t[:]).then_inc(wld_sem, 16)
            x_tile(0)
            sy.dma_start(out=srel_sb[:, :ca],
                         in_=srel_t[:, :ca]).then_inc(srl_sem, 16)
            sy.dma_start(out=iota_sb[:], in_=iota_t[:]).then_inc(io_sem, 16)
            if nxt > 1:
                x_tile(1)
            sy.dma_start(out=bias_sb[:], in_=bias_t[:]).then_inc(bs_sem, 16)
            if nxt > 2:
                x_tile(2)
            if nxt > 3:
                x_tile(3)
            if ca < nbp * 2:
                sy.dma_start(out=srel_sb[:, ca:],
                             in_=srel_t[:, ca:]).then_inc(srlb_sem, 16)
            for T in range(NXB, nxt):
                lc = min((T - NXB + 1) * CH, nch) - 1
                sy.wait_ge(pe_sem, pe_pos_mm_hi[lc])
                x_tile(T)

        @block.vector
        def _(ve):
            ve.wait_ge(io_sem, 16)
            ve.wait_ge(srl_sem, 16)
            for j in range(nbatch):
                if j == SRA:
                    ve.wait_ge(srlb_sem, 16)
                if batch_blocker[j] >= 0:
                    ve.wait_ge(pe_sem, pe_pos_ind[batch_blocker[j]])
                wb = batch_w[j]
                in0 = bass.AP(iota_sb.ap().tensor, 0,
                              [[512, P], [0, SGB], [1, wb]])
                in1 = bass.AP(srel_sb.ap().tensor, j * SGB * 2,
                              [[nbp * 2, P], [2, SGB], [0, wb // 2], [1, 2]])
                out = bass.AP(s_sb.ap().tensor, batch_off[j],
                              [[SRING, P], [wb, SGB], [1, wb]])
                ve.tensor_tensor(out=out, in0=in0, in1=in1,
                                 op=mybir.AluOpType.is_equal
                                 ).then_inc(dve_sem, 1)

        @block.tensor
        def _(pe):
            pe.wait_ge(wld_sem, 32)
            for (op, idx) in pe_seq:
                if op == "MM":
                    c = idx
                    a = c // BCH
                    T = c // CH
                    if c % CH == 0:
                        pe.wait_ge(x_sems[T % NXB], 16 * (T // NXB + 1))
                    if c % BCH == 0 and a >= 2:
                        pe.wait_ge(act_sem, act_pos_cp[a - 2])
                    q = c % BCH
                    qlast = min((a + 1) * BCH, nch) - a * BCH - 1
                    b, k = chunks[c][0], chunks[c][1]
                    xoff = (c % CH) * 2 * P
                    pe.matmul(out=cps[:, a % 2, q, :],
                              lhsT=x_sb[:, T % NXB, xoff:xoff + P],
                              rhs=w_sb[:, (k * 2) * c_out:(k * 2 + 1) * c_out],
                              start=(q == 0), stop=False)
                    pe.matmul(out=cps[:, a % 2, q, :],
                              lhsT=x_sb[:, T % NXB, xoff + P:xoff + 2 * P],
                              rhs=w_sb[:, (k * 2 + 1) * c_out:(k * 2 + 2) * c_out],
                              start=False, stop=(q == qlast)).then_inc(pe_sem, 1)
                elif op == "W":
                    actv, dvev = idx
                    pe.wait_ge(act_sem, actv)
                    pe.wait_ge(dve_sem, dvev)
                elif op == "Z":
                    b, bank = idx
                    if b >= 1:
                        pe.wait_ge(act_sem, act_pos_relu_bank[(b - 1, bank)])
                    pe.matmul(out=ops[:, bank, 0:512],
                              lhsT=w_sb[:, 0:P], rhs=zero_sb[:, 0:512],
                              start=True, stop=False).then_inc(pe_sem, 1)
                else:
                    g = idx
                    ci, bank, rmin, w = groups[g]
                    b = chunks[ci][0]
                    col = rmin - bank * 512
                    sc0 = int(g_scol[g])
                    pe.matmul(out=ops[:, bank, col:col + w],
                              lhsT=c_sb[:, a % NCSB,
                                        (ci % BCH) * c_out:(ci % BCH + 1) * c_out],
                              rhs=s_sb[:, sc0:sc0 + w],
                              start=False,
                              stop=(last[(b, bank)] == g),
                              ).then_inc(pe_sem, 1)

        @block.scalar
        def _(sc):
            sc.wait_ge(bs_sem, 16)
            for e in act_seq:
                if e[0] == "CP":
                    a = e[1]
                    n = min((a + 1) * BCH, nch) - a * BCH
                    sc.wait_ge(pe_sem, pe_pos_mm_hi[a * BCH + n - 1])
                    sc.copy(out=c_sb[:, a % NCSB, :n * c_out],
                            in_=cps[:, a % 2, 0:n, :]).then_inc(act_sem, 1)
                elif e[0] == "RELU":
                    _, b, bank = e
                    sc.wait_ge(pe_sem, pe_pos_ind[last[(b, bank)]])
                    if b >= 2:
                        sc.wait_ge(out_sems[b % 2], 16 * (b // 2))
                    w0 = bank * 512
                    w1 = min(BW, w0 + 512)
                    sc.activation(out=o_sb[:, b % 2, w0:w1],
                                  in_=ops[:, bank, 0:w1 - w0],
                                  func=mybir.ActivationFunctionType.Relu,
                                  bias=bias_sb[:, 0:1], scale=1.0
                                  ).then_inc(act_sem, 1)
                else:
                    b = e[1]
                    sc.dma_start(out=outT[:, b * BW:(b + 1) * BW],
                                 in_=o_sb[:, b % 2, :]
                                 ).then_inc(out_sems[b % 2], 16)
                    # dma_start counts in act_seq positions via act_sem? no:
                    # OUT entries do not inc act_sem; positions above account
                    # for that (act_pos built over all entries but waits use
                    # only CP/RELU positions).

    nc.compile()
    return nc


# ---------------------------------------------------------------------------
# entry
# ---------------------------------------------------------------------------

_CACHE = {}


def kernel(feats, weight, bias, in_map, out_map, n_out):
    from concourse.bass_utils import run_bass_kernel_spmd

    feats = np.asarray(feats, dtype=np.float32)
    weight = np.asarray(weight, dtype=np.float32)
    bias = np.asarray(bias, dtype=np.float32)
    in_map = np.asarray(in_map)
    out_map = np.asarray(out_map)
    n_out = int(n_out)
    n_in = feats.shape[0]
    K = weight.shape[0]

    sched = build_schedule(in_map, out_map, n_in, n_out, N_CORES)
    in_maps = make_inputs(feats, weight, bias, sched)

    key = (n_in, n_out, K, sched["nch"], sched["ngroups"])
    nc = _CACHE.get(key)
    if nc is None:
        nc = build_program(sched)
        _CACHE[key] = nc

    res = run_bass_kernel_spmd(nc, in_maps, list(range(N_CORES)))
    rpc_eff = sched["rpc_eff"]
    outs = []
    for c in range(N_CORES):
        r = min(rpc_eff, n_out - c * rpc_eff)
        ot = res.results[c]["outT"]              # [128, rpc] bf16
        outs.append(np.asarray(ot[:, :r], dtype=np.float32).T)
    return np.ascontiguousarray(np.concatenate(outs, 0))


# revision 32
# speedup vs baseline: 1.1535x; 1.0103x over previous
"""MinkowskiEngine deconv+ReLU v3: breakpoint row-windows + span-packed S.

Per core (output-partitioned, rows [c*rpc_eff, ...)):
  host sorts the core's K*M/8 pairs by (block, k, out-row). For each
  (block, k) bucket, shared row BREAKPOINTS close a window when any core
  would exceed 128 pairs; chunk = (b, k, window) holds <=128 pairs per
  core (k-pure, row-range-pure). X = feats rows per chunk slot,
  pre-transposed to lhsT layout, streamed sequentially (no gather).

Device: chunk GEMM (bf16) -> contrib PSUM -> ACT copy to SBUF bf16 ->
  indicator matmuls (lhsT=contrib, rhs=0/1 S) accumulate out^T tiles in
  PSUM -> ACT bias+ReLU -> out DMA (ACT queue). S matrices are built
  span-packed: one DVE/GpSimd is_equal op per batch of SGB groups, each
  group's columns sized to the batch max width (vs per-tile 128-wide
  instances) -- ~40% less vector-engine work.

Structural schedule (chunk/group counts) shared by all 8 cores; only
tensor contents differ (SPMD single program).
"""
import numpy as np
from contextlib import ExitStack

import concourse.bass as bass
import concourse.bacc as bacc
from concourse import mybir

P = 128
GG = 14          # tiles per block (block = PSUM-resident out^T stripe)
CH = 32          # chunks per X DMA tile (2 MB)
NXB = 4          # x stream buffers
NCSB = 4         # contrib sbuf ring (bank batches)
BCH = 4          # chunks per contrib PSUM bank batch
SGB = 16         # groups per S build batch
SORT_WIN = 16    # width-sort window; = SGB keeps build order == consume
                 # order (wider windows overflow the S ring -> deadlock)
SRING = 16384    # S ring columns (bf16)
N_CORES = 8

# ---------------------------------------------------------------------------
# host schedule
# ---------------------------------------------------------------------------


def build_schedule(in_map, out_map, n_in, n_out, n_cores=N_CORES):
    K, M = in_map.shape
    rpc_eff = -(-n_out // n_cores)
    n_tiles = -(-rpc_eff // P)
    assert n_tiles % GG == 0, (n_tiles, GG)
    nblk = n_tiles // GG
    rpc = n_tiles * P
    R = GG * P                        # rows per block (1792)

    kk = np.repeat(np.arange(K, dtype=np.int64), M)
    irow = in_map.astype(np.int64).ravel()
    orow = out_map.astype(np.int64).ravel()
    core = np.minimum(orow // rpc_eff, n_cores - 1)

    percore = []                      # (kc, ic, oc) sorted by (blk, k, oc)
    cnt = np.zeros((n_cores, nblk, K), np.int64)
    for c in range(n_cores):
        m = core == c
        kc, ic, oc = kk[m], irow[m], orow[m] - c * rpc_eff
        blk = oc // R
        order = np.lexsort((oc, kc, blk))
        percore.append((kc[order], ic[order], oc[order]))
        np.add.at(cnt[c], (blk, kc), 1)

    # bucket start offsets in each core's sorted pair list
    bstart = np.zeros((n_cores, nblk * K), np.int64)
    for c in range(n_cores):
        np.cumsum(cnt[c].reshape(-1)[:-1], out=bstart[c, 1:])
    bstart = bstart.reshape(n_cores, nblk, K)

    # shared row breakpoints per (b, k): close window when any core would
    # exceed P pairs. windows[b][k] = list of (r_lo, r_hi) block-local rows.
    # chunks table in processing order (b, r_lo, k).
    chunks = []                       # (b, k, r_lo, r_hi, [per-core slices])
    for b in range(nblk):
        blk_chunks = []
        for k in range(K):
            # per-core row lists for this bucket
            rows_c = []
            for c in range(n_cores):
                s0 = bstart[c, b, k]
                rows_c.append(percore[c][2][s0:s0 + cnt[c, b, k]] - b * R)
            n_c = [len(r) for r in rows_c]
            pos = [0] * n_cores
            while any(pos[c] < n_c[c] for c in range(n_cores)):
                # shared row frontier: the row of the (P+1)-th pending pair,
                # minimized over cores. Each core then takes up to P pairs
                # with row <= frontier (boundary-row pairs may split across
                # windows, keeping the binding core exactly full).
                r_hi = R
                for c in range(n_cores):
                    rem = n_c[c] - pos[c]
                    if rem > P:
                        r_hi = min(r_hi, int(rows_c[c][pos[c] + P]))
                ends = []
                w_lo, w_hi = R, 0
                for c in range(n_cores):
                    e = pos[c] + min(P, int(np.searchsorted(
                        rows_c[c][pos[c]:], r_hi, side="right")))
                    if e > pos[c]:
                        w_lo = min(w_lo, int(rows_c[c][pos[c]]))
                        w_hi = max(w_hi, int(rows_c[c][e - 1]) + 1)
                    ends.append(e)
                assert w_hi > 0
                blk_chunks.append((b, k, w_lo, w_hi,
                                   [(pos[c], ends[c]) for c in range(n_cores)]))
                pos = ends
        blk_chunks.sort(key=lambda t: (t[2], t[1]))
        chunks.extend(blk_chunks)
    nch = len(chunks)

    # groups: one per chunk (PSUM writes may cross bank boundaries --
    # HW-verified), split only if the union span exceeds MAXW
    MAXW = 1000
    groups = []                       # (ci, rmin, w)
    gsrel = []                        # per group: list over cores of
                                      # (slot_positions, srel_values)
    for ci, (b, k, r_lo, r_hi, sl) in enumerate(chunks):
        nseg = -(-(r_hi - r_lo) // MAXW)
        for si in range(nseg):
            lo = r_lo + (r_hi - r_lo) * si // nseg
            hi = r_lo + (r_hi - r_lo) * (si + 1) // nseg
            rmin, rmax = 1 << 30, -1
            percore_part = []
            for c in range(n_cores):
                s0 = bstart[c, b, k]
                p0, p1 = sl[c]
                rows = percore[c][2][s0 + p0:s0 + p1] - b * R
                m = (rows >= lo) & (rows < hi)
                if m.any():
                    rr = rows[m]
                    rmin = min(rmin, int(rr.min()))
                    rmax = max(rmax, int(rr.max()))
                percore_part.append(m)
            if rmax < 0:
                continue
            w = rmax - rmin + 1
            if w & 1:                 # even width for the srel x2 AP trick
                if rmax + 1 < GG * P:
                    w += 1
                else:                 # at block end: extend left instead
                    rmin -= 1
                    w += 1
            gs = []
            for c in range(n_cores):
                s0 = bstart[c, b, k]
                p0, p1 = sl[c]
                rows = percore[c][2][s0 + p0:s0 + p1] - b * R
                m = percore_part[c]
                gs.append((np.nonzero(m)[0], rows[m] - rmin))
            groups.append((ci, rmin, w))
            gsrel.append(gs)
    ngroups = len(groups)

    # S build batches: SGB groups of similar width from a SORT_WIN sliding
    # window (cuts pad-to-max waste); srel columns stored in BUILD order so
    # each batch reads a contiguous srel2 slice. Ring allocation with wrap;
    # per-batch blocker = max group index whose IND must complete first
    # (pe_pos_ind is monotone in group index).
    build_order = []
    for w0 in range(0, ngroups, SORT_WIN):
        idx = sorted(range(w0, min(w0 + SORT_WIN, ngroups)),
                     key=lambda g: groups[g][2])
        build_order.extend(idx)
    nbatch = -(-ngroups // SGB)
    batch_groups = [build_order[j * SGB:(j + 1) * SGB] for j in range(nbatch)]
    batch_w = [max(groups[g][2] for g in bg) for bg in batch_groups]
    batch_off = []
    batch_blocker = []                # group index that must be consumed
    placed = []                       # (start, end, last_group)
    off = 0
    for j in range(nbatch):
        wb = batch_w[j]
        sz = SGB * wb
        assert sz <= SRING, (j, wb)
        if off + sz > SRING:
            off = 0
        s, e = off, off + sz
        blocker = -1
        for (ps, pe_, lg) in placed:
            if ps < e and s < pe_:
                blocker = max(blocker, lg)
        placed = [(ps, pe_, lg) for (ps, pe_, lg) in placed if lg > blocker]
        placed.append((s, e, max(batch_groups[j])))
        batch_off.append(off)
        batch_blocker.append(blocker)
        off = e

    # group -> batch, S column start, srel build position
    g_batch = np.empty(ngroups, np.int64)
    g_scol = np.empty(ngroups, np.int64)
    g_bp = np.empty(ngroups, np.int64)
    for j, bg in enumerate(batch_groups):
        for i, g in enumerate(bg):
            g_batch[g] = j
            g_scol[g] = batch_off[j] + i * batch_w[j]
            g_bp[g] = j * SGB + i

    # PSUM accumulation bookkeeping per (block, bank) over TOUCHED banks
    # (a group's write may span several 512-col banks)
    def gbanks(g):
        _, rmin, w = groups[g]
        return range(rmin // 512, (rmin + w - 1) // 512 + 1)

    first = {}
    last = {}
    for g, (ci, rmin, w) in enumerate(groups):
        b = chunks[ci][0]
        for bank in gbanks(g):
            key = (b, bank)
            if key not in first:
                first[key] = g
            last[key] = g
    for b in range(nblk):
        for t in range(4):
            assert (b, t) in first, ("uncovered bank", b, t)

    grp_of_chunk = [[] for _ in range(nch)]
    for g, (ci, rmin, w) in enumerate(groups):
        grp_of_chunk[ci].append(g)

    g_banks = [list(gbanks(g)) for g in range(ngroups)]

    return dict(chunks=chunks, groups=groups, gsrel=gsrel, g_scol=g_scol,
                g_banks=g_banks, g_batch=g_batch, g_bp=g_bp,
                batch_w=batch_w, batch_off=batch_off,
                batch_blocker=batch_blocker, nbatch=nbatch,
                first=first, last=last, grp_of_chunk=grp_of_chunk,
                nch=nch, ngroups=ngroups, nblk=nblk, n_tiles=n_tiles,
                rpc=rpc, rpc_eff=rpc_eff, K=K,
                percore=percore, bstart=bstart, cnt=cnt)


# ---------------------------------------------------------------------------
# input packing
# ---------------------------------------------------------------------------


def make_inputs(feats, weight, bias, sched):
    import ml_dtypes
    bf16 = ml_dtypes.bfloat16
    K = sched["K"]
    nch, ngroups = sched["nch"], sched["ngroups"]
    chunks, groups, gsrel = sched["chunks"], sched["groups"], sched["gsrel"]
    percore, bstart = sched["percore"], sched["bstart"]
    c_in = feats.shape[1]
    c_out = weight.shape[2]
    assert c_in == 256 and c_out == 128

    f16 = feats.astype(bf16)
    wd = np.ascontiguousarray(
        weight.astype(bf16).reshape(K, 2, P, c_out).transpose(2, 0, 1, 3)
    ).reshape(P, K * 2 * c_out)
    iota = np.tile(np.arange(512, dtype=np.int16), (P, 1)).copy()
    zd = np.zeros((P, 512), np.uint16)
    biasd = np.tile(bias.astype(np.float32)[:, None], (1, 8))

    nxt = -(-nch // CH)

    in_maps = []
    for c in range(N_CORES):
        kc, ic, oc = percore[c]
        gidx = np.zeros((nch, P), np.int64)
        for ci, (b, k, r_lo, r_hi, sl) in enumerate(chunks):
            s0 = bstart[c, b, k]
            p0, p1 = sl[c]
            n = p1 - p0
            if n:
                gidx[ci, :n] = ic[s0 + p0:s0 + p1]
        # X: [nxt*128, CH*256]; [T*128+p, q*256+h*128+j] = f16[gidx[c,j], h*128+p]
        A = f16[gidx]                                  # [nch, j(P), 256]
        A = A.reshape(nch, P, 2, P).transpose(0, 3, 2, 1)   # [c, p, h, j]
        xpad = np.zeros((nxt * CH, P, 2, P), bf16)
        xpad[:nch] = A
        xd = np.ascontiguousarray(
            xpad.reshape(nxt, CH, P, 2, P).transpose(0, 2, 1, 3, 4)
        ).reshape(nxt * P, CH * 2 * P)
        # srel2: [P, 2*nbp] int16 in BUILD order; cols 2bp,2bp+1 = row
        # offset of the pair within its group (or -3000)
        g_bp = sched["g_bp"]
        nbp = sched["nbatch"] * SGB
        srel = np.full((nbp, P), -3000, np.int64)
        for g in range(ngroups):
            slot_pos, vals = gsrel[g][c]
            if len(slot_pos):
                srel[g_bp[g], slot_pos] = vals
        srel2 = np.repeat(srel.T.astype(np.int16), 2, axis=1)
        in_maps.append(dict(xd=xd.view(np.uint16), wd=wd.view(np.uint16),
                            srel2=srel2, iota=iota, biasd=biasd, zd=zd))
    return in_maps


# ---------------------------------------------------------------------------
# device program
# ---------------------------------------------------------------------------


def build_program(sched):
    K = sched["K"]
    nch, ngroups, nblk = sched["nch"], sched["ngroups"], sched["nblk"]
    chunks, groups = sched["chunks"], sched["groups"]
    g_scol, g_batch = sched["g_scol"], sched["g_batch"]
    batch_w, batch_off = sched["batch_w"], sched["batch_off"]
    batch_blocker, nbatch = sched["batch_blocker"], sched["nbatch"]
    first, last = sched["first"], sched["last"]
    grp_of_chunk = sched["grp_of_chunk"]
    g_banks = sched["g_banks"]
    rpc = sched["rpc"]
    c_out = 128
    nxt = -(-nch // CH)
    nbat = -(-nch // BCH)
    nbp = nbatch * SGB
    BW = GG * P                    # block out^T width (1792)

    # ACT: per batch a: CP(a); RELU(b, bank) as soon as that bank's last IND
    # group has been emitted; out DMA of block b after its last RELU.
    relu_bat = {}
    for (b, bank), g in last.items():
        relu_bat[(b, bank)] = min(groups[g][0] // BCH + 1, nbat)
    blk_last_bank = {}               # block -> (bat, bank) of its last relu
    act_seq = []
    for a in range(nbat + 1):
        if a < nbat:
            act_seq.append(("CP", a))
        ready = sorted((b, bank) for (b, bank), rb in relu_bat.items()
                       if rb == a)
        done_blocks = []
        for (b, bank) in ready:
            act_seq.append(("RELU", b, bank))
            blk_last_bank.setdefault(b, 0)
            blk_last_bank[b] += 1
            if blk_last_bank[b] == 4:
                done_blocks.append(b)
        for b in sorted(done_blocks):
            act_seq.append(("OUT", b))
    act_pos = {}
    pos = 0
    for e in act_seq:
        if e[0] != "OUT":            # OUT does not inc act_sem
            pos += 1
        act_pos[e] = pos
    act_pos_cp = {e[1]: act_pos[e] for e in act_seq if e[0] == "CP"}
    act_pos_relu_bank = {}           # (block, bank) -> pos
    for e in act_seq:
        if e[0] == "RELU":
            act_pos_relu_bank[(e[1], e[2])] = act_pos[e]


    # ---- symbolic engine sequences for position bookkeeping ----
    # PE: per batch a: MMs (2 per chunk), then merged INDs of batch a-1
    pe_seq = []
    for a in range(nbat + 1):
        if a < nbat:
            for c in range(a * BCH, min((a + 1) * BCH, nch)):
                pe_seq.append(("MM", c))           # one entry = 2 matmuls
        if a >= 1:
            lo, hi = (a - 1) * BCH, min(a * BCH, nch)
            batch_grps = [g for ci in range(lo, hi) for g in grp_of_chunk[ci]]
            if batch_grps:
                # hoisted waits: one dve + one act wait for the whole IND
                # run so the PE reorder window can pull LDWEIGHTS ahead of
                # in-flight matmuls (per-matmul waits serialize LDW).
                # act value covers the CP and any Z's relu-free precondition
                # (act_sem is monotone).
                zkeys = []
                for g in batch_grps:
                    bg_ = chunks[groups[g][0]][0]
                    for bank in g_banks[g]:
                        if first[(bg_, bank)] == g and bg_ >= 1:
                            zkeys.append((bg_ - 1, bank))
                actv = max([act_pos_cp[a - 1]] +
                           [act_pos_relu_bank[zk] for zk in zkeys])
                dvev = max(int(g_batch[g]) + 1 for g in batch_grps)
                pe_seq.append(("W", (actv, dvev)))
            for ci in range(lo, hi):
                for g in grp_of_chunk[ci]:
                    bg_ = chunks[groups[g][0]][0]
                    for bank in g_banks[g]:
                        if first[(bg_, bank)] == g:
                            pe_seq.append(("Z", (bg_, bank)))
                    pe_seq.append(("IND", g))
    # sparse pe_sem inc points: per-matmul then_inc costs NX issue slots and
    # blocks LDWEIGHTS pull-ahead. Inc only at: last MM of each BCH batch,
    # bank-last INDs (RELU preconditions), and last IND of each run.
    run_last = set()
    prev = None
    for e in pe_seq:
        if e[0] == "IND":
            prev = e[1]
        elif e[0] == "MM" and prev is not None:
            run_last.add(prev)
            prev = None
    if prev is not None:
        run_last.add(prev)
    pe_inc = []                       # parallel to pe_seq: bool
    for e in pe_seq:
        if e[0] == "MM":
            c = e[1]
            pe_inc.append(c % BCH == BCH - 1 or c == nch - 1)
        elif e[0] == "IND":
            g = e[1]
            bg_ = chunks[groups[g][0]][0]
            pe_inc.append(any(last[(bg_, bank)] == g for bank in g_banks[g])
                          or g in run_last)
        else:
            pe_inc.append(False)
    # value after each entry = #incs so far; waiters use the value at the
    # NEXT inc-point at-or-after the event (reverse pass)
    cum = []
    v = 0
    for inc in pe_inc:
        if inc:
            v += 1
        cum.append(v)
    nxt_cov = [0] * len(pe_seq)
    cover = v
    for i in range(len(pe_seq) - 1, -1, -1):
        if pe_inc[i]:
            cover = cum[i]
        nxt_cov[i] = cover
    pe_pos_mm_hi = {}                # chunk -> covering inc value
    pe_pos_ind = {}                  # group -> covering inc value
    for i, e in enumerate(pe_seq):
        if e[0] == "MM":
            pe_pos_mm_hi[e[1]] = nxt_cov[i]
        elif e[0] == "IND":
            pe_pos_ind[e[1]] = nxt_cov[i]

    # S-batch needed by IND group g: batch(g); builder blocker positions
    # SRA: batches covered by the first srel dma
    SRA = max(1, min(nbatch, 64))

    nc = bacc.Bacc("TRN2", target_bir_lowering=False, debug=False)
    xd_t = nc.dram_tensor("xd", [nxt * P, CH * 2 * P], mybir.dt.uint16,
                          kind="ExternalInput").ap()
    wd_t = nc.dram_tensor("wd", [P, K * 2 * c_out], mybir.dt.uint16,
                          kind="ExternalInput").ap()
    srel_t = nc.dram_tensor("srel2", [P, nbp * 2], mybir.dt.int16,
                            kind="ExternalInput").ap()
    iota_t = nc.dram_tensor("iota", [P, 512], mybir.dt.int16,
                            kind="ExternalInput").ap()
    bias_t = nc.dram_tensor("biasd", [P, 8], mybir.dt.float32,
                            kind="ExternalInput").ap()
    zd_t = nc.dram_tensor("zd", [P, 512], mybir.dt.uint16,
                          kind="ExternalInput").ap()
    outT = nc.dram_tensor("outT", [P, rpc], mybir.dt.bfloat16,
                          kind="ExternalOutput").ap()

    with ExitStack() as stack:
        block = stack.enter_context(nc.Block())
        wld_sem = stack.enter_context(nc.semaphore("wld"))
        srl_sem = stack.enter_context(nc.semaphore("srl"))
        srlb_sem = stack.enter_context(nc.semaphore("srlb"))
        io_sem = stack.enter_context(nc.semaphore("io"))
        bs_sem = stack.enter_context(nc.semaphore("bs"))
        x_sems = [stack.enter_context(nc.semaphore(f"x{i}"))
                  for i in range(NXB)]
        pe_sem = stack.enter_context(nc.semaphore("pe"))
        act_sem = stack.enter_context(nc.semaphore("act"))
        dve_sem = stack.enter_context(nc.semaphore("dve"))
        out_sems = [stack.enter_context(nc.semaphore(f"out{i}"))
                    for i in range(2)]

        # PSUM: ops banks 0-3 (out^T stripe), cps banks 4,5 (contrib)
        ops = stack.enter_context(
            nc.psum_tensor("ops", [P, 4 * 512], mybir.dt.float32))
        cps = stack.enter_context(
            nc.psum_tensor("cps", [P, 2, BCH, c_out], mybir.dt.float32))

        x_sb = stack.enter_context(
            nc.sbuf_tensor("x_sb", [P, NXB, CH * 2 * P], mybir.dt.bfloat16))
        w_sb = stack.enter_context(
            nc.sbuf_tensor("w_sb", [P, K * 2 * c_out], mybir.dt.bfloat16))
        srel_sb = stack.enter_context(
            nc.sbuf_tensor("srel_sb", [P, nbp * 2], mybir.dt.int16))
        iota_sb = stack.enter_context(
            nc.sbuf_tensor("iota_sb", [P, 512], mybir.dt.int16))
        bias_sb = stack.enter_context(
            nc.sbuf_tensor("bias_sb", [P, 8], mybir.dt.float32))
        zero_sb = stack.enter_context(
            nc.sbuf_tensor("zero_sb", [P, 512], mybir.dt.bfloat16))
        s_sb = stack.enter_context(
            nc.sbuf_tensor("s_sb", [P, SRING], mybir.dt.bfloat16))
        c_sb = stack.enter_context(
            nc.sbuf_tensor("c_sb", [P, NCSB, BCH * c_out], mybir.dt.bfloat16))
        o_sb = stack.enter_context(
            nc.sbuf_tensor("o_sb", [P, 2, BW], mybir.dt.bfloat16))

        @block.sync
        def _(sy):
            def x_tile(T):
                sy.dma_start(out=x_sb[:, T % NXB, :].bitcast(mybir.dt.uint16),
                             in_=xd_t[T * P:(T + 1) * P, :]
                             ).then_inc(x_sems[T % NXB], 16)

            ca = SRA * SGB * 2
            sy.dma_start(out=w_sb[:].bitcast(mybir.dt.uint16),
                         in_=wd_t[:]).then_inc(wld_sem, 16)
            sy.dma_start(out=zero_sb[:].bitcast(mybir.dt.uint16),
                         in_=zd_t[:]).then_inc(wld_sem, 16)
            x_tile(0)
            sy.dma_start(out=srel_sb[:, :ca],
                         in_=srel_t[:, :ca]).then_inc(srl_sem, 16)
            sy.dma_start(out=iota_sb[:], in_=iota_t[:]).then_inc(io_sem, 16)
            if nxt > 1:
                x_tile(1)
            sy.dma_start(out=bias_sb[:], in_=bias_t[:]).then_inc(bs_sem, 16)
            if nxt > 2:
                x_tile(2)
            if nxt > 3:
                x_tile(3)
            if ca < nbp * 2:
                sy.dma_start(out=srel_sb[:, ca:],
                             in_=srel_t[:, ca:]).then_inc(srlb_sem, 16)
            for T in range(NXB, nxt):
                lc = min((T - NXB + 1) * CH, nch) - 1
                sy.wait_ge(pe_sem, pe_pos_mm_hi[lc])
                x_tile(T)

        @block.vector
        def _(ve):
            ve.wait_ge(io_sem, 16)
            ve.wait_ge(srl_sem, 16)
            for j in range(nbatch):
                if j == SRA:
                    ve.wait_ge(srlb_sem, 16)
                if batch_blocker[j] >= 0:
                    ve.wait_ge(pe_sem, pe_pos_ind[batch_blocker[j]])
                wb = batch_w[j]
                in0 = bass.AP(iota_sb.ap().tensor, 0,
                              [[512, P], [0, SGB], [1, wb]])
                in1 = bass.AP(srel_sb.ap().tensor, j * SGB * 2,
                              [[nbp * 2, P], [2, SGB], [0, wb // 2], [1, 2]])
                out = bass.AP(s_sb.ap().tensor, batch_off[j],
                              [[SRING, P], [wb, SGB], [1, wb]])
                ve.tensor_tensor(out=out, in0=in0, in1=in1,
                                 op=mybir.AluOpType.is_equal
                                 ).then_inc(dve_sem, 1)

        @block.tensor
        def _(pe):
            pe.wait_ge(wld_sem, 32)
            for i_e, (op, idx) in enumerate(pe_seq):
                if op == "MM":
                    c = idx
                    a = c // BCH
                    T = c // CH
                    if c % CH == 0:
                        pe.wait_ge(x_sems[T % NXB], 16 * (T // NXB + 1))
                    if c % BCH == 0 and a >= 2:
                        pe.wait_ge(act_sem, act_pos_cp[a - 2])
                    q = c % BCH
                    qlast = min((a + 1) * BCH, nch) - a * BCH - 1
                    b, k = chunks[c][0], chunks[c][1]
                    xoff = (c % CH) * 2 * P
                    pe.matmul(out=cps[:, a % 2, q, :],
                              lhsT=x_sb[:, T % NXB, xoff:xoff + P],
                              rhs=w_sb[:, (k * 2) * c_out:(k * 2 + 1) * c_out],
                              start=(q == 0), stop=False)
                    mm = pe.matmul(out=cps[:, a % 2, q, :],
                                   lhsT=x_sb[:, T % NXB, xoff + P:xoff + 2 * P],
                                   rhs=w_sb[:, (k * 2 + 1) * c_out:(k * 2 + 2) * c_out],
                                   start=False, stop=(q == qlast))
                    if q == qlast:
                        mm.then_inc(pe_sem, 1)
                elif op == "W":
                    actv, dvev = idx
                    pe.wait_ge(act_sem, actv)
                    pe.wait_ge(dve_sem, dvev)
                elif op == "Z":
                    b, bank = idx
                    if b >= 1:
                        pe.wait_ge(act_sem, act_pos_relu_bank[(b - 1, bank)])
                    pe.matmul(out=ops[:, bank * 512:(bank + 1) * 512],
                              lhsT=w_sb[:, 0:P], rhs=zero_sb[:, 0:512],
                              start=True, stop=False, skip_group_check=True)
                else:
                    g, inc = idx, pe_inc[i_e]
                    ci, rmin, w = groups[g]
                    a = ci // BCH
                    sc0 = int(g_scol[g])
                    mm = pe.matmul(out=ops[:, rmin:rmin + w],
                                   lhsT=c_sb[:, a % NCSB,
                                             (ci % BCH) * c_out:(ci % BCH + 1) * c_out],
                                   rhs=s_sb[:, sc0:sc0 + w],
                                   start=False, stop=False,
                                   skip_group_check=True)
                    if inc:
                        mm.then_inc(pe_sem, 1)

        @block.scalar
        def _(sc):
            sc.wait_ge(bs_sem, 16)
            for e in act_seq:
                if e[0] == "CP":
                    a = e[1]
                    n = min((a + 1) * BCH, nch) - a * BCH
                    sc.wait_ge(pe_sem, pe_pos_mm_hi[a * BCH + n - 1])
                    sc.copy(out=c_sb[:, a % NCSB, :n * c_out],
                            in_=cps[:, a % 2, 0:n, :]).then_inc(act_sem, 1)
                elif e[0] == "RELU":
                    _, b, bank = e
                    sc.wait_ge(pe_sem, pe_pos_ind[last[(b, bank)]])
                    if b >= 2:
                        sc.wait_ge(out_sems[b % 2], 16 * (b // 2))
                    w0 = bank * 512
                    w1 = min(BW, w0 + 512)
                    sc.activation(out=o_sb[:, b % 2, w0:w1],
                                  in_=ops[:, w0:w1],
                                  func=mybir.ActivationFunctionType.Relu,
                                  bias=bias_sb[:, 0:1], scale=1.0
                                  ).then_inc(act_sem, 1)
                else:
                    b = e[1]
                    sc.dma_start(out=outT[:, b * BW:(b + 1) * BW],
                                 in_=o_sb[:, b % 2, :]
                                 ).then_inc(out_sems[b % 2], 16)
                    # dma_start counts in act_seq positions via act_sem? no:
                    # OUT entries do not inc act_sem; positions above account
                    # for that (act_pos built over all entries but waits use
                    # only CP/RELU positions).

    nc.compile()
    return nc


# ---------------------------------------------------------------------------
# entry
# ---------------------------------------------------------------------------

_CACHE = {}


def kernel(feats, weight, bias, in_map, out_map, n_out):
    from concourse.bass_utils import run_bass_kernel_spmd

    feats = np.asarray(feats, dtype=np.float32)
    weight = np.asarray(weight, dtype=np.float32)
    bias = np.asarray(bias, dtype=np.float32)
    in_map = np.asarray(in_map)
    out_map = np.asarray(out_map)
    n_out = int(n_out)
    n_in = feats.shape[0]
    K = weight.shape[0]

    sched = build_schedule(in_map, out_map, n_in, n_out, N_CORES)
    in_maps = make_inputs(feats, weight, bias, sched)

    key = (n_in, n_out, K, sched["nch"], sched["ngroups"])
    nc = _CACHE.get(key)
    if nc is None:
        nc = build_program(sched)
        _CACHE[key] = nc

    res = run_bass_kernel_spmd(nc, in_maps, list(range(N_CORES)))
    rpc_eff = sched["rpc_eff"]
    outs = []
    for c in range(N_CORES):
        r = min(rpc_eff, n_out - c * rpc_eff)
        ot = res.results[c]["outT"]              # [128, rpc] bf16
        outs.append(np.asarray(ot[:, :r], dtype=np.float32).T)
    return np.ascontiguousarray(np.concatenate(outs, 0))
